# revision 4
# baseline (speedup 1.0000x reference)
"""DKVMN scatter_memory kernel for 8 Trainium2 NeuronCores.

Math: the reference scan only reads the (B, M, Dv) memory through
read @ Wf_r, so the recurrence collapses to

  pred[t,b] = cq[q[t,b]] + bf + sum_{s<t} cv[w[s,b]] * <S[q[t,b]], S[q[s,b]]>

with S = softmax(Eq@Wa + ba) (100 x 32), cq = Eq@Wf[:64], cv = Ev@Wf[64:],
w = (2q + a) % 100.  Both gathered quantities depend only on the input
pair p = q*100 + a, so the host precomputes a (10000, 72) f16 table
  row(p) = [ S[q] (32) | S[q]*cv[w(q,a)] (32) | cq[q]+bf (1) | pad ]
(weights-only preprocessing).  Per core (batch-sharded, Bs=128) the
device computes p = q*100+a, gathers the 16384 rows via an indirect
DMA straight from HBM into (t, (b, elem)) layout, then
  P = Lstrict @ v   (one f16 matmul per 32-b chunk, PSUM f32)
  pred = sum_m A*P + cq
"""
import functools
import numpy as np

import concourse.bass as bass
import concourse.bacc as bacc
import concourse.mybir as mybir
from concourse import tile
from concourse.bass_utils import run_bass_kernel_spmd

T, B, M, DQ, DV, VOCAB = 128, 1024, 32, 64, 64, 100
NCORES = 8
BS = B // NCORES   # 128
PAIRS = VOCAB * VOCAB
ROW = 72           # table row length in f16 elems (65 used, padded)
STR = 80           # slab row stride (non-contiguous runs => per-row descriptors)
NCH = 4            # b-chunks per core
CB = BS // NCH     # 32 b per chunk
F32 = mybir.dt.float32
F16 = mybir.dt.float16
I32 = mybir.dt.int32
AX = mybir.AxisListType
OP = mybir.AluOpType


def _build():
    nc = bacc.Bacc("TRN2", num_devices=NCORES, debug=False, target_bir_lowering=False)
    qTB = nc.dram_tensor("qTB", [T, BS], I32, kind="ExternalInput").ap()
    aTB = nc.dram_tensor("aTB", [T, BS], I32, kind="ExternalInput").ap()
    tab = nc.dram_tensor("tab", [PAIRS, ROW], F16, kind="ExternalInput").ap()
    usT = nc.dram_tensor("usT", [T, T], F16, kind="ExternalInput").ap()
    preds = nc.dram_tensor("preds", [T, BS], F32, kind="ExternalOutput").ap()

    with tile.TileContext(nc) as tc:
        with (
            tc.tile_pool(name="sb", bufs=1) as sb,
            tc.tile_pool(name="ps", bufs=1, space="PSUM") as ps,
        ):
            q_t = sb.tile([T, BS], I32)
            a_t = sb.tile([T, BS], I32)
            us_t = sb.tile([T, T], F16)
            nc.sync.dma_start(q_t[:], qTB[:])
            nc.sync.dma_start(a_t[:], aTB[:])
            nc.scalar.dma_start(us_t[:], usT[:])

            idx = sb.tile([T, BS], I32)
            nc.vector.scalar_tensor_tensor(
                out=idx[:], in0=q_t[:], scalar=100, in1=a_t[:],
                op0=OP.mult, op1=OP.add)

            slab = sb.tile([T, BS * STR], F16)
            slab3 = slab[:].rearrange("p (b e) -> p b e", e=STR)
            out_sb = sb.tile([T, BS], F32)

            GB = 512 // M  # 16 b per matmul (PSUM bank limit: 512 f32)
            for ch in range(NCH):
                bsl = slice(ch * CB, (ch + 1) * CB)
                nc.gpsimd.indirect_dma_start(
                    out=slab3[:, bsl, 0:ROW],
                    out_offset=None,
                    in_=tab[:],
                    in_offset=bass.IndirectOffsetOnAxis(ap=idx[:, bsl], axis=0),
                )
                for h in range(CB // GB):
                    gsl = slice(ch * CB + h * GB, ch * CB + (h + 1) * GB)
                    pP = ps.tile([T, GB * M], F32, tag=f"pP{h}")
                    nc.tensor.matmul(pP[:], us_t[:], slab3[:, gsl, M:2 * M],
                                     start=True, stop=True)
                    pT = sb.tile([T, GB * M], F16)
                    nc.scalar.copy(pT[:], pP[:])
                    apv = sb.tile([T, GB * M], F16)
                    nc.vector.tensor_tensor(
                        apv[:].rearrange("p (b m) -> p b m", m=M),
                        slab3[:, gsl, 0:M],
                        pT[:].rearrange("p (b m) -> p b m", m=M),
                        OP.mult)
                    red = sb.tile([T, GB], F32)
                    nc.vector.tensor_reduce(
                        red[:], apv[:].rearrange("p (b m) -> p b m", m=M),
                        AX.X, OP.add)
                    nc.vector.tensor_tensor(
                        out_sb[:, gsl], red[:],
                        slab3[:, gsl, 2 * M:2 * M + 1].rearrange("p b e -> p (b e)"),
                        OP.add)

            nc.sync.dma_start(preds[:], out_sb[:])

    nc.compile()
    return nc


@functools.lru_cache(maxsize=1)
def _get_nc():
    return _build()


def _host_consts(Eq, Ev, Wa, ba, Wf, bf):
    Eq = np.asarray(Eq, np.float32)
    Ev = np.asarray(Ev, np.float32)
    Wa = np.asarray(Wa, np.float32)
    ba = np.asarray(ba, np.float32).reshape(M)
    Wf = np.asarray(Wf, np.float32).reshape(DQ + DV)
    bf = np.asarray(bf, np.float32).reshape(1)[0]

    logits = Eq @ Wa + ba                     # (100, 32)
    e = np.exp(logits - logits.max(axis=1, keepdims=True))
    S = e / e.sum(axis=1, keepdims=True)      # (100, 32)
    cq = Eq @ Wf[:DQ] + bf                    # (100,)
    cv = Ev @ Wf[DQ:]                         # (100,)

    qg = np.arange(PAIRS) // VOCAB            # (10000,)
    ag = np.arange(PAIRS) % VOCAB
    w = (2 * qg + ag) % VOCAB
    tabf = np.zeros((PAIRS, ROW), np.float32)
    tabf[:, 0:M] = S[qg]
    tabf[:, M:2 * M] = S[qg] * cv[w][:, None]
    tabf[:, 2 * M] = cq[qg]
    return {
        "tab": tabf.astype(np.float16),
        "usT": np.triu(np.ones((T, T), np.float32), k=1).astype(np.float16),
    }


def kernel(questions, answers, Eq, Ev, Wa, ba, Wf, bf):
    questions = np.asarray(questions)
    answers = np.asarray(answers)
    consts = _host_consts(Eq, Ev, Wa, ba, Wf, bf)
    nc = _get_nc()
    in_maps = []
    for c in range(NCORES):
        sl = slice(c * BS, (c + 1) * BS)
        m = dict(consts)
        m["qTB"] = np.ascontiguousarray(questions[:, sl]).astype(np.int32)
        m["aTB"] = np.ascontiguousarray(answers[:, sl]).astype(np.int32)
        in_maps.append(m)
    res = run_bass_kernel_spmd(nc, in_maps, list(range(NCORES)))
    preds = np.concatenate([res.results[c]["preds"] for c in range(NCORES)], axis=1)
    return preds.astype(np.float32)


# revision 9
# speedup vs baseline: 1.3627x; 1.3627x over previous
"""DKVMN scatter_memory kernel for 8 Trainium2 NeuronCores.

Math: the reference scan only reads the (B, M, Dv) memory through
read @ Wf_r, so the recurrence collapses to

  pred[t,b] = cq[q[t,b]] + bf + sum_{s<t} cv[w[s,b]] * <S[q[t,b]], S[q[s,b]]>

with S = softmax(Eq@Wa + ba) (100 x 32), cq = Eq@Wf[:64], cv = Ev@Wf[64:],
w = (2q + a) % 100.  Both gathered quantities depend only on the input
pair p = q*100 + a, so the host precomputes a (10000, 128) f16 table
  row(p) = [ S[q] (32) | S[q]*cv[w(q,a)] (32) | cq[q]+bf (1) | pad ]
(weights-only preprocessing).  Per core (batch-sharded, Bs=128) the
device computes p = q*100+a and gathers the 16384 rows from HBM with
SWDGE dma_gather directly into (t=tok%128, (b=tok//128, elem)) layout,
then
  P = Lstrict @ v   (f16 matmuls, PSUM f32)
  pred = sum_m A*P + cq

The gather's wrapped index layout (token k at partition k%16 + 16g for
all replicas g, free k//16) is a pure reshuffle of the question/answer
tensors, done host-side; the q*100+a arithmetic runs on-device.
"""
import functools
import numpy as np

import concourse.bass as bass
import concourse.bacc as bacc
import concourse.mybir as mybir
from concourse import tile
from concourse.bass_utils import run_bass_kernel_spmd

T, B, M, DQ, DV, VOCAB = 128, 1024, 32, 64, 64, 100
NCORES = 8
BS = B // NCORES   # 128
N = T * BS         # tokens per core = 16384
PAIRS = VOCAB * VOCAB
ROW = 128          # table row length in f16 elems (256B, SWDGE minimum)
NCH = 4            # gather chunks (b-ranges)
CB = BS // NCH     # 32 b per chunk
WCOL = N // 16     # wrapped index columns = 1024
F32 = mybir.dt.float32
F16 = mybir.dt.float16
I32 = mybir.dt.int32
I16 = mybir.dt.int16
AX = mybir.AxisListType
OP = mybir.AluOpType


def _build():
    nc = bacc.Bacc("TRN2", num_devices=NCORES, debug=False,
                   target_bir_lowering=False, num_swdge_queues=1)
    qW = nc.dram_tensor("qW", [T, WCOL], I32, kind="ExternalInput").ap()
    aW = nc.dram_tensor("aW", [T, WCOL], I32, kind="ExternalInput").ap()
    tab = nc.dram_tensor("tab", [PAIRS, ROW], F16, kind="ExternalInput").ap()
    usT = nc.dram_tensor("usT", [T, T], F16, kind="ExternalInput").ap()
    preds = nc.dram_tensor("preds", [T, BS], F32, kind="ExternalOutput").ap()

    with tile.TileContext(nc) as tc:
        with (
            tc.tile_pool(name="sb", bufs=1) as sb,
            tc.tile_pool(name="ps", bufs=1, space="PSUM") as ps,
        ):
            qw_t = sb.tile([T, WCOL], I32)
            aw_t = sb.tile([T, WCOL], I32)
            us_t = sb.tile([T, T], F16)
            nc.sync.dma_start(qw_t[:], qW[:])
            nc.scalar.dma_start(aw_t[:], aW[:])
            nc.scalar.dma_start(us_t[:], usT[:])

            # pair index p = q*100 + a, already in wrapped layout
            idxw32 = sb.tile([T, WCOL], I32)
            nc.vector.scalar_tensor_tensor(
                out=idxw32[:], in0=qw_t[:], scalar=100, in1=aw_t[:],
                op0=OP.mult, op1=OP.add)
            idxw = sb.tile([T, WCOL], I16)
            nc.vector.tensor_copy(idxw[:], idxw32[:])

            slab = sb.tile([T, BS * ROW], F16)
            slab3 = slab[:].rearrange("p (b e) -> p b e", e=ROW)
            out_sb = sb.tile([T, BS], F32)

            GB = 512 // M  # 16 b per matmul (PSUM bank limit: 512 f32)
            for ch in range(NCH):
                bsl = slice(ch * CB, (ch + 1) * CB)
                nc.gpsimd.dma_gather(
                    out_ap=slab3[:, bsl, :],
                    in_ap=tab[:],
                    idxs_ap=idxw[:, ch * (CB * 8):(ch + 1) * (CB * 8)],
                    num_idxs=CB * T,
                    num_idxs_reg=CB * T,
                    elem_size=ROW,
                    queue_num=0,
                )
                for h in range(CB // GB):
                    gsl = slice(ch * CB + h * GB, ch * CB + (h + 1) * GB)
                    pP = ps.tile([T, GB * M], F32, tag=f"pP{h}")
                    nc.tensor.matmul(pP[:], us_t[:], slab3[:, gsl, M:2 * M],
                                     start=True, stop=True)
                    pT = sb.tile([T, GB * M], F16)
                    nc.scalar.copy(pT[:], pP[:])
                    apv = sb.tile([T, GB * M], F16)
                    nc.vector.tensor_tensor(
                        apv[:].rearrange("p (b m) -> p b m", m=M),
                        slab3[:, gsl, 0:M],
                        pT[:].rearrange("p (b m) -> p b m", m=M),
                        OP.mult)
                    red = sb.tile([T, GB], F32)
                    nc.vector.tensor_reduce(
                        red[:], apv[:].rearrange("p (b m) -> p b m", m=M),
                        AX.X, OP.add)
                    nc.vector.tensor_tensor(
                        out_sb[:, gsl], red[:],
                        slab3[:, gsl, 2 * M:2 * M + 1].rearrange("p b e -> p (b e)"),
                        OP.add)

            nc.sync.dma_start(preds[:], out_sb[:])

    nc.compile()
    return nc


@functools.lru_cache(maxsize=1)
def _get_nc():
    return _build()


def _host_consts(Eq, Ev, Wa, ba, Wf, bf):
    Eq = np.asarray(Eq, np.float32)
    Ev = np.asarray(Ev, np.float32)
    Wa = np.asarray(Wa, np.float32)
    ba = np.asarray(ba, np.float32).reshape(M)
    Wf = np.asarray(Wf, np.float32).reshape(DQ + DV)
    bf = np.asarray(bf, np.float32).reshape(1)[0]

    logits = Eq @ Wa + ba                     # (100, 32)
    e = np.exp(logits - logits.max(axis=1, keepdims=True))
    S = e / e.sum(axis=1, keepdims=True)      # (100, 32)
    cq = Eq @ Wf[:DQ] + bf                    # (100,)
    cv = Ev @ Wf[DQ:]                         # (100,)

    qg = np.arange(PAIRS) // VOCAB            # (10000,)
    ag = np.arange(PAIRS) % VOCAB
    w = (2 * qg + ag) % VOCAB
    tabf = np.zeros((PAIRS, ROW), np.float32)
    tabf[:, 0:M] = S[qg]
    tabf[:, M:2 * M] = S[qg] * cv[w][:, None]
    tabf[:, 2 * M] = cq[qg]
    return {
        "tab": tabf.astype(np.float16),
        "usT": np.triu(np.ones((T, T), np.float32), k=1).astype(np.float16),
    }


def _wrap(x):
    """(T, BS) int -> SWDGE wrapped+replicated (128, 1024) int32.

    Token k = b*128 + t goes to partition k%16 (+16g for g=0..7),
    free position k//16 = b*8 + t//16.
    """
    x = np.asarray(x).astype(np.int32)         # (T=128, BS=128), t-major
    # w16[r, b*8 + j] = x[16*j + r, b]
    w16 = x.reshape(8, 16, BS).transpose(1, 2, 0).reshape(16, WCOL)
    return np.tile(w16, (8, 1))                # replicate to 128 partitions


def kernel(questions, answers, Eq, Ev, Wa, ba, Wf, bf):
    questions = np.asarray(questions)
    answers = np.asarray(answers)
    consts = _host_consts(Eq, Ev, Wa, ba, Wf, bf)
    nc = _get_nc()
    in_maps = []
    for c in range(NCORES):
        sl = slice(c * BS, (c + 1) * BS)
        m = dict(consts)
        m["qW"] = _wrap(questions[:, sl])
        m["aW"] = _wrap(answers[:, sl])
        in_maps.append(m)
    res = run_bass_kernel_spmd(nc, in_maps, list(range(NCORES)))
    preds = np.concatenate([res.results[c]["preds"] for c in range(NCORES)], axis=1)
    return preds.astype(np.float32)


# revision 10
# speedup vs baseline: 2.2336x; 1.6391x over previous
"""DKVMN scatter_memory kernel for 8 Trainium2 NeuronCores.

Math: the reference scan only reads the (B, M, Dv) memory through
read @ Wf_r, so the recurrence collapses to

  pred[t,b] = cq[q[t,b]] + bf + sum_{s<t} cv[w[s,b]] * <S[q[t,b]], S[q[s,b]]>

with S = softmax(Eq@Wa + ba) (100 x 32), cq = Eq@Wf[:64], cv = Ev@Wf[64:],
w = (2q + a) % 100.

Host prep: weight tables [S | cq+bf] (100x33) and cv broadcast (100x32),
plus an indicator (one-hot) encoding of the integer inputs q and w
(100 x 32768 f16, grouped so each 16-batch group's columns are one
contiguous 800KB chunk).  Device per core (batch-sharded, Bs=128):
stream the one-hot chunks over 3 DMA queues; per b, three small f16
matmuls gather A = S[q], cq[q], R = cv[w] via PE; then per 16-b group
  P = Lstrict @ (A*R)   (f16 matmul, PSUM f32)
  pred = sum_m A*P + cq
"""
import functools
import numpy as np

import concourse.bass as bass
import concourse.bacc as bacc
import concourse.mybir as mybir
from concourse import tile
from concourse.bass_utils import run_bass_kernel_spmd

T, B, M, DQ, DV, VOCAB = 128, 1024, 32, 64, 64, 100
NCORES = 8
BS = B // NCORES   # 128
N = T * BS         # tokens per core = 16384
NG = 8             # b-groups
GB = BS // NG      # 16 b per group
GCOL = 2 * GB * T  # one-hot cols per group (q-half then w-half) = 4096
F32 = mybir.dt.float32
F16 = mybir.dt.float16
AX = mybir.AxisListType
OP = mybir.AluOpType


def _build():
    nc = bacc.Bacc("TRN2", num_devices=NCORES, debug=False, target_bir_lowering=False)
    oh = nc.dram_tensor("oh", [VOCAB, 2 * N], F16, kind="ExternalInput").ap()
    scq = nc.dram_tensor("scq", [VOCAB, M + 1], F16, kind="ExternalInput").ap()
    cvr = nc.dram_tensor("cvr", [VOCAB, M], F16, kind="ExternalInput").ap()
    usT = nc.dram_tensor("usT", [T, T], F16, kind="ExternalInput").ap()
    preds = nc.dram_tensor("preds", [T, BS], F32, kind="ExternalOutput").ap()

    with tile.TileContext(nc) as tc:
        with (
            tc.tile_pool(name="sb", bufs=1) as sb,
            tc.tile_pool(name="ps", bufs=2, space="PSUM") as ps,
        ):
            scq_t = sb.tile([VOCAB, M + 1], F16)
            cvr_t = sb.tile([VOCAB, M], F16)
            us_t = sb.tile([T, T], F16)
            nc.sync.dma_start(scq_t[:], scq[:])
            nc.sync.dma_start(cvr_t[:], cvr[:])
            nc.sync.dma_start(us_t[:], usT[:])

            oh_t = sb.tile([VOCAB, 2 * N], F16)
            engs = [nc.sync, nc.scalar, nc.gpsimd]
            for g in range(NG):
                sl = slice(g * GCOL, (g + 1) * GCOL)
                engs[g % 3].dma_start(oh_t[:, sl], oh[:, sl])

            out_sb = sb.tile([T, BS], F32)

            for g in range(NG):
                gsl = slice(g * GB, (g + 1) * GB)
                pA = ps.tile([T, GB * M], F32, tag="pA")
                pC = ps.tile([T, GB], F32, tag="pC")
                pR = ps.tile([T, GB * M], F32, tag="pR")
                for k in range(GB):
                    qc = g * GCOL + k * T
                    wc = g * GCOL + GB * T + k * T
                    ohq = oh_t[:, qc:qc + T]
                    ohw = oh_t[:, wc:wc + T]
                    nc.tensor.matmul(pA[:, k * M:(k + 1) * M], ohq,
                                     scq_t[:, 0:M], start=True, stop=True)
                    nc.tensor.matmul(pC[:, k:k + 1], ohq,
                                     scq_t[:, M:M + 1], start=True, stop=True)
                    nc.tensor.matmul(pR[:, k * M:(k + 1) * M], ohw,
                                     cvr_t[:], start=True, stop=True)
                a_g = sb.tile([T, GB * M], F16, tag="a_g")
                c_g = sb.tile([T, GB], F16, tag="c_g")
                nc.scalar.copy(a_g[:], pA[:])
                nc.scalar.copy(c_g[:], pC[:])
                v_g = sb.tile([T, GB * M], F16, tag="v_g")
                nc.vector.tensor_tensor(v_g[:], a_g[:], pR[:], OP.mult)
                pP = ps.tile([T, GB * M], F32, tag="pP")
                nc.tensor.matmul(pP[:], us_t[:], v_g[:], start=True, stop=True)
                p_g = sb.tile([T, GB * M], F16, tag="p_g")
                nc.scalar.copy(p_g[:], pP[:])
                ap_g = sb.tile([T, GB * M], F16, tag="ap_g")
                nc.vector.tensor_tensor(ap_g[:], a_g[:], p_g[:], OP.mult)
                red = sb.tile([T, GB], F32, tag="red")
                nc.vector.tensor_reduce(
                    red[:], ap_g[:].rearrange("p (b m) -> p b m", m=M),
                    AX.X, OP.add)
                nc.vector.tensor_tensor(out_sb[:, gsl], red[:], c_g[:], OP.add)

            nc.sync.dma_start(preds[:], out_sb[:])

    nc.compile()
    return nc


@functools.lru_cache(maxsize=1)
def _get_nc():
    return _build()


def _host_consts(Eq, Ev, Wa, ba, Wf, bf):
    Eq = np.asarray(Eq, np.float32)
    Ev = np.asarray(Ev, np.float32)
    Wa = np.asarray(Wa, np.float32)
    ba = np.asarray(ba, np.float32).reshape(M)
    Wf = np.asarray(Wf, np.float32).reshape(DQ + DV)
    bf = np.asarray(bf, np.float32).reshape(1)[0]

    logits = Eq @ Wa + ba                     # (100, 32)
    e = np.exp(logits - logits.max(axis=1, keepdims=True))
    S = e / e.sum(axis=1, keepdims=True)      # (100, 32)
    cq = Eq @ Wf[:DQ] + bf                    # (100,)
    cv = Ev @ Wf[DQ:]                         # (100,)

    scq = np.concatenate([S, cq[:, None]], axis=1)      # (100, 33)
    cvr = np.repeat(cv[:, None], M, axis=1)             # (100, 32)
    return {
        "scq": scq.astype(np.float16),
        "cvr": cvr.astype(np.float16),
        "usT": np.triu(np.ones((T, T), np.float32), k=1).astype(np.float16),
    }


_IOTA = np.arange(VOCAB, dtype=np.int32)[:, None]


def _onehot_cols(q, w):
    """q, w: (T, GB) int -> one-hot (100, 2*GB*T) f16, cols (b-major, t)."""
    qc = q.T.reshape(1, -1)                    # (1, GB*T) b-major
    wc = w.T.reshape(1, -1)
    block = np.concatenate([qc, wc], axis=1)   # (1, 2*GB*T)
    return (block == _IOTA).astype(np.float16)


def kernel(questions, answers, Eq, Ev, Wa, ba, Wf, bf):
    questions = np.asarray(questions).astype(np.int32)
    answers = np.asarray(answers).astype(np.int32)
    w_all = (2 * questions + answers) % VOCAB
    consts = _host_consts(Eq, Ev, Wa, ba, Wf, bf)
    nc = _get_nc()
    in_maps = []
    for c in range(NCORES):
        sl = slice(c * BS, (c + 1) * BS)
        qs, ws = questions[:, sl], w_all[:, sl]
        ohc = np.concatenate(
            [_onehot_cols(qs[:, g * GB:(g + 1) * GB], ws[:, g * GB:(g + 1) * GB])
             for g in range(NG)], axis=1)
        m = dict(consts)
        m["oh"] = ohc
        in_maps.append(m)
    res = run_bass_kernel_spmd(nc, in_maps, list(range(NCORES)))
    preds = np.concatenate([res.results[c]["preds"] for c in range(NCORES)], axis=1)
    return preds.astype(np.float32)


# revision 11
# speedup vs baseline: 2.7952x; 1.2515x over previous
"""DKVMN scatter_memory kernel for 8 Trainium2 NeuronCores.

Math: the reference scan only reads the (B, M, Dv) memory through
read @ Wf_r, so the recurrence collapses to

  pred[t,b] = cq[q[t,b]] + bf + sum_{s<t} cv[w[s,b]] * <S[q[t,b]], S[q[s,b]]>

with S = softmax(Eq@Wa + ba) (100 x 32), cq = Eq@Wf[:64], cv = Ev@Wf[64:],
w = (2q + a) % 100.

Host prep: weight tables [S | cq+bf] (100x33) and cv broadcast (100x32),
plus an indicator (one-hot) encoding of the integer inputs q and w
(100 x 32768 fp8, exact 0/1), chunked so the two HWDGE queues stream it
in four 0.8MB pieces.  Device per core (batch-sharded, Bs=128): per b,
three small matmuls (fp8 one-hot x f16 table) gather A = S[q], cq[q],
R = cv[w] on the PE; then per 16-b group
  P = Lstrict @ (A*R)   (f16 matmul, PSUM f32)
  pred = sum_m A*P + cq
"""
import functools
import numpy as np
import ml_dtypes

import concourse.bass as bass
import concourse.bacc as bacc
import concourse.mybir as mybir
from concourse import tile
from concourse.bass_utils import run_bass_kernel_spmd

T, B, M, DQ, DV, VOCAB = 128, 1024, 32, 64, 64, 100
NCORES = 8
BS = B // NCORES   # 128
N = T * BS         # tokens per core = 16384
NG = 8             # b-groups
GB = BS // NG      # 16 b per group
GCOL = 2 * GB * T  # one-hot cols per group (q-half then w-half) = 4096
NCHK = 4           # load chunks (2 groups each)
F32 = mybir.dt.float32
F16 = mybir.dt.float16
F8 = mybir.dt.float8e4
AX = mybir.AxisListType
OP = mybir.AluOpType


def _build():
    nc = bacc.Bacc("TRN2", num_devices=NCORES, debug=False, target_bir_lowering=False)
    oh = nc.dram_tensor("oh", [VOCAB, 2 * N], F8, kind="ExternalInput").ap()
    scq = nc.dram_tensor("scq", [VOCAB, M + 1], F16, kind="ExternalInput").ap()
    cvr = nc.dram_tensor("cvr", [VOCAB, M], F16, kind="ExternalInput").ap()
    usT = nc.dram_tensor("usT", [T, T], F16, kind="ExternalInput").ap()
    preds = nc.dram_tensor("preds", [T, BS], F32, kind="ExternalOutput").ap()

    with tile.TileContext(nc) as tc:
        with (
            tc.tile_pool(name="sb", bufs=1) as sb,
            tc.tile_pool(name="ps", bufs=2, space="PSUM") as ps,
        ):
            scq_t = sb.tile([VOCAB, M + 1], F16)
            cvr_t = sb.tile([VOCAB, M], F16)
            us_t = sb.tile([T, T], F16)
            nc.sync.dma_start(scq_t[:], scq[:])
            nc.sync.dma_start(cvr_t[:], cvr[:])
            nc.sync.dma_start(us_t[:], usT[:])

            CC = 2 * GCOL  # cols per chunk = 8192
            oh_c = [sb.tile([VOCAB, CC], F8, name=f"ohc{c}") for c in range(NCHK)]
            for c in range(NCHK):
                eng = nc.sync if c % 2 == 0 else nc.scalar
                eng.dma_start(oh_c[c][:], oh[:, c * CC:(c + 1) * CC])

            out_sb = sb.tile([T, BS], F32)

            for g in range(NG):
                gsl = slice(g * GB, (g + 1) * GB)
                ohg = oh_c[g // 2]
                base = (g % 2) * GCOL
                pA = ps.tile([T, GB * M], F32, tag="pA")
                pC = ps.tile([T, GB], F32, tag="pC")
                pR = ps.tile([T, GB * M], F32, tag="pR")
                for k in range(GB):
                    qc = base + k * T
                    nc.tensor.matmul(pA[:, k * M:(k + 1) * M], ohg[:, qc:qc + T],
                                     scq_t[:, 0:M], start=True, stop=True)
                a_g = sb.tile([T, GB * M], F16, tag="a_g")
                nc.scalar.copy(a_g[:], pA[:])
                for k in range(GB):
                    qc = base + k * T
                    nc.tensor.matmul(pC[:, k:k + 1], ohg[:, qc:qc + T],
                                     scq_t[:, M:M + 1], start=True, stop=True)
                c_g = sb.tile([T, GB], F16, tag="c_g")
                nc.scalar.copy(c_g[:], pC[:])
                for k in range(GB):
                    wc = base + GB * T + k * T
                    nc.tensor.matmul(pR[:, k * M:(k + 1) * M], ohg[:, wc:wc + T],
                                     cvr_t[:], start=True, stop=True)
                r_g = sb.tile([T, GB * M], F16, tag="r_g")
                nc.scalar.copy(r_g[:], pR[:])
                v_g = sb.tile([T, GB * M], F16, tag="v_g")
                nc.vector.tensor_tensor(v_g[:], a_g[:], r_g[:], OP.mult)
                pP = ps.tile([T, GB * M], F32, tag="pP")
                nc.tensor.matmul(pP[:], us_t[:], v_g[:], start=True, stop=True)
                ap_g = sb.tile([T, GB * M], F16, tag="ap_g")
                nc.vector.tensor_tensor(ap_g[:], a_g[:], pP[:], OP.mult)
                red = sb.tile([T, GB], F32, tag="red")
                nc.vector.tensor_reduce(
                    red[:], ap_g[:].rearrange("p (b m) -> p b m", m=M),
                    AX.X, OP.add)
                nc.vector.tensor_tensor(out_sb[:, gsl], red[:], c_g[:], OP.add)

            nc.sync.dma_start(preds[:], out_sb[:])

    nc.compile()
    return nc


@functools.lru_cache(maxsize=1)
def _get_nc():
    return _build()


def _host_consts(Eq, Ev, Wa, ba, Wf, bf):
    Eq = np.asarray(Eq, np.float32)
    Ev = np.asarray(Ev, np.float32)
    Wa = np.asarray(Wa, np.float32)
    ba = np.asarray(ba, np.float32).reshape(M)
    Wf = np.asarray(Wf, np.float32).reshape(DQ + DV)
    bf = np.asarray(bf, np.float32).reshape(1)[0]

    logits = Eq @ Wa + ba                     # (100, 32)
    e = np.exp(logits - logits.max(axis=1, keepdims=True))
    S = e / e.sum(axis=1, keepdims=True)      # (100, 32)
    cq = Eq @ Wf[:DQ] + bf                    # (100,)
    cv = Ev @ Wf[DQ:]                         # (100,)

    scq = np.concatenate([S, cq[:, None]], axis=1)      # (100, 33)
    cvr = np.repeat(cv[:, None], M, axis=1)             # (100, 32)
    return {
        "scq": scq.astype(np.float16),
        "cvr": cvr.astype(np.float16),
        "usT": np.triu(np.ones((T, T), np.float32), k=1).astype(np.float16),
    }


_IOTA = np.arange(VOCAB, dtype=np.int32)[:, None]


def _onehot_cols(q, w):
    """q, w: (T, GB) int -> one-hot (100, 2*GB*T) fp8, cols (b-major, t)."""
    qc = q.T.reshape(1, -1)                    # (1, GB*T) b-major
    wc = w.T.reshape(1, -1)
    block = np.concatenate([qc, wc], axis=1)   # (1, 2*GB*T)
    return (block == _IOTA).astype(ml_dtypes.float8_e4m3fn)


def kernel(questions, answers, Eq, Ev, Wa, ba, Wf, bf):
    questions = np.asarray(questions).astype(np.int32)
    answers = np.asarray(answers).astype(np.int32)
    w_all = (2 * questions + answers) % VOCAB
    consts = _host_consts(Eq, Ev, Wa, ba, Wf, bf)
    nc = _get_nc()
    in_maps = []
    for c in range(NCORES):
        sl = slice(c * BS, (c + 1) * BS)
        qs, ws = questions[:, sl], w_all[:, sl]
        ohc = np.concatenate(
            [_onehot_cols(qs[:, g * GB:(g + 1) * GB], ws[:, g * GB:(g + 1) * GB])
             for g in range(NG)], axis=1)
        m = dict(consts)
        m["oh"] = ohc
        in_maps.append(m)
    res = run_bass_kernel_spmd(nc, in_maps, list(range(NCORES)))
    preds = np.concatenate([res.results[c]["preds"] for c in range(NCORES)], axis=1)
    return preds.astype(np.float32)
